# revision 40
# baseline (speedup 1.0000x reference)
"""Causal GQA attention on 8 TRN2 NeuronCores (Bass/Tile).

Problem: B=2, Tq=Tk=2048, Hq=32, Hkv=8, D=128, fp32, causal softmax(QK^T/sqrt(D))V.

Sharding (tensor parallel over heads): core c handles q-heads [4c, 4c+4) and
kv-head c. Per core that is 8 independent (batch, head) single-head attentions
of Q[2048,128] x K[2048,128] -> V[2048,128]. No cross-core communication.

Per-core kernel layout (host pre-arranges everything into PE-friendly layouts):
  qT   [8, 128, 2048]  f32r  per (b, h_local): Q^T        (d on partitions)
  kT   [2, 128, 2048]  f32r  per batch:        K^T        (d on partitions)
  vp   [2, 128, 16*129] f16  per batch: V tiled [kt, 128, 129] partition-major,
                             col 128 of each tile = 1.0 (fused row-sum column)
  mk   [128, 128]      f16   upper-triangular (c >= p) 0/1 mask for diag tiles
  out  [8, 128, 16*128] f32  per pair: out[qrow, qt*128 + d]

Algorithm per (pair, q-chunk of 512) with k-tile pairs merged into [128,1024]
PSUM tiles (2 banks) so each ACT exp instruction covers 2 k-tiles:
  S^T[kt] = kT[kt-tile].T @ qT-chunk      (fp32r matmul, N=512, full PE rate)
  P^T[kt] = exp(SCALE * S^T[kt])          (ACT, fp16 out; causal: skip columns
                                           left of the diagonal, mask the
                                           diagonal 128x128 block on DVE)
  out[qs] = sum_kt P^T[kt][:, qs-block].T @ Vp[kt]   (fp16 matmul, N=129,
                                           PSUM-accumulated; col 128 = rowsum)
  out_norm = out[:, :128] * reciprocal(out[:, 128])  (DVE)

The Scalar engine (exp) is the bottleneck at ~83% of the kernel span, so the
whole schedule is built around keeping it saturated: PV matmul groups are
queued and drained between QK tile-pairs at a paced rate, so the PE always
alternates between feeding ACT new S^T tiles and chewing through PV backlog.
"""

from collections import deque

import numpy as np

import concourse.bass as bass
import concourse.tile as tile
from concourse import bacc, mybir
from concourse.bass_utils import run_bass_kernel_spmd

B, T, HQ, HKV, D = 2, 2048, 32, 8, 128
SCALE = 0.08838834764831845  # 1/sqrt(128)
NCORES = 8
HL = HQ // NCORES        # q heads per core
PAIRS = B * HL           # (batch, local head) pairs per core
QCH = 512                # q chunk = matmul moving free dim
NQC = T // QCH           # q chunks
KTS = T // 128           # k tiles
f32 = mybir.dt.float32
f32r = mybir.dt.float32r
f16 = mybir.dt.float16

_CACHE = {}


def _build():
    nc = bacc.Bacc("TRN2", target_bir_lowering=False, debug=False, num_devices=NCORES)
    qT = nc.dram_tensor("qT", [PAIRS, 128, T], f16, kind="ExternalInput").ap()
    kT = nc.dram_tensor("kT", [B, 128, T], f16, kind="ExternalInput").ap()
    vp = nc.dram_tensor("vp", [B, 128, KTS * 129], f16, kind="ExternalInput").ap()
    mk = nc.dram_tensor("mk", [128, 128], f16, kind="ExternalInput").ap()
    out = nc.dram_tensor("out", [PAIRS, 128, KTS * 128], f32, kind="ExternalOutput").ap()
    EXP = mybir.ActivationFunctionType.Exp

    # Total QK tile-pair slots and PV groups, for pacing the PV drain.
    slots_total = PAIRS * sum((4 * qc + 4) // 2 for qc in range(NQC))   # 160
    groups_total = PAIRS * NQC * 4                                       # 128

    with tile.TileContext(nc) as tc:
        with tc.tile_pool(name="const", bufs=1) as cst, \
             tc.tile_pool(name="sb", bufs=2) as sbp, \
             tc.tile_pool(name="pt", bufs=16) as ptp, \
             tc.tile_pool(name="st", bufs=3, space="PSUM") as stp, \
             tc.tile_pool(name="po", bufs=2, space="PSUM") as pop:
            # Latency-critical first loads ride the HWDGE sync queue;
            # bulk constants and output stores ride the gpsimd queue.
            kt0a = cst.tile([128, 256], f16, name="ktc0a", tag="ktc0a")
            nc.scalar.dma_start(out=kt0a[:], in_=kT[0][:, 0:256])
            mk_sb = cst.tile([128, 128], f16, name="mask", tag="mask")
            nc.sync.dma_start(out=mk_sb[:], in_=mk[:])
            # kT[0] is split across three tiles loaded in need-order, so the
            # first QK matmuls only wait on the small head pieces (tile-
            # granularity dependency tracking would otherwise stall them on
            # the full 1MB load).
            kt0b = cst.tile([128, 768], f16, name="ktc0b", tag="ktc0b")
            kt0c = cst.tile([128, 1024], f16, name="ktc0c", tag="ktc0c")
            vp0 = cst.tile([128, KTS * 129], f16, name="vpc0", tag="vpc0")
            kt1 = cst.tile([128, T], f16, name="ktc1", tag="ktc1")
            vp1 = cst.tile([128, KTS * 129], f16, name="vpc1", tag="vpc1")
            vps = [vp0, vp1]

            def ksrc(b, kt):
                if b == 1:
                    return kt1[:, kt * 128:(kt + 1) * 128]
                if kt < 2:
                    return kt0a[:, kt * 128:(kt + 1) * 128]
                if kt < 8:
                    return kt0b[:, (kt - 2) * 128:(kt - 1) * 128]
                return kt0c[:, (kt - 8) * 128:(kt - 7) * 128]

            def emit_late_consts(qc):
                # Bulk constant loads feed the gpsimd queue during pair 0,
                # in the order the pipeline needs them.
                if qc == 0:
                    nc.gpsimd.dma_start(out=kt0b[:], in_=kT[0][:, 256:1024])
                elif qc == 1:
                    nc.gpsimd.dma_start(out=kt0c[:], in_=kT[0][:, 1024:2048])
                    nc.gpsimd.dma_start(out=vp0[:], in_=vp[0])
                elif qc == 2:
                    nc.gpsimd.dma_start(out=kt1[:], in_=kT[1])
                else:
                    nc.gpsimd.dma_start(out=vp1[:], in_=vp[1])


            def pv_group(pair, b, qc, ptmap, ost, qs):
                gq = 4 * qc + qs
                ops = pop.tile([128, 129], f32, name="ops", tag="ops")
                for kt in range(gq + 1):
                    pt, h = ptmap[kt]
                    nc.tensor.matmul(
                        ops[:],
                        pt[:, h * 512 + qs * 128:h * 512 + (qs + 1) * 128],
                        vps[b][:, kt * 129:(kt + 1) * 129],
                        start=(kt == 0), stop=(kt == gq),
                    )
                rs = sbp.tile([128, 1], f32, name="rs", tag="rs", bufs=6)
                nc.vector.reciprocal(rs[:], ops[:, 128:129])
                nc.vector.tensor_scalar_mul(
                    ost[:, qs * 128:(qs + 1) * 128], ops[:, 0:128], rs[:]
                )
                if pair == PAIRS - 1 and qc == NQC - 1:
                    # final chunk: store each 128-col block as it normalizes
                    # so the stores overlap the remaining PV groups
                    nc.sync.dma_start(
                        out=out[pair][:, gq * 128:(gq + 1) * 128],
                        in_=ost[:, qs * 128:(qs + 1) * 128],
                    )
                elif qs == 3:
                    nc.sync.dma_start(
                        out=out[pair][:, qc * QCH:(qc + 1) * QCH], in_=ost[:]
                    )

            def emit_qk_exp(b, q_sb, qc, chunk_id):
                # k-tile pairs share one [128,1024] PSUM tile so one ACT exp
                # covers 2 k-tiles. The 4 partial (diagonal-crossing) k-tiles
                # are paired in reverse offset order -- (j3,j0) and (j2,j1),
                # low-offset tile in the second half -- so the single exp
                # range [c0:1024) wastes only 128 dead columns per chunk.
                nkt = 4 * qc + 4
                if qc == 0:
                    kt_pairs = [(3, 0), (2, 1)]
                    c0s = [384, 256]
                else:
                    kt_pairs = [(2 * a, 2 * a + 1) for a in range(2 * qc)] + \
                        [(4 * qc + 3, 4 * qc), (4 * qc + 2, 4 * qc + 1)]
                    c0s = [0] * (2 * qc) + [384, 256]
                ptmap = {}
                for a, (kta, ktb) in enumerate(kt_pairs):
                    st = stp.tile([128, 1024], f32, name="st", tag="st")
                    for h, kt in enumerate((kta, ktb)):
                        nc.tensor.matmul(
                            st[:, h * 512:(h + 1) * 512],
                            ksrc(b, kt),
                            q_sb[:],
                            start=True, stop=True,
                        )
                    pt = ptp.tile([128, 1024], f16, name="pt", tag="pt")
                    nc.scalar.activation(
                        pt[:, c0s[a]:1024], st[:, c0s[a]:1024], EXP, scale=SCALE
                    )
                    for h, kt in enumerate((kta, ktb)):
                        j = kt - 4 * qc
                        if j >= 0:   # diagonal 128x128 mask
                            o = h * 512 + 128 * j
                            nc.vector.tensor_mul(
                                pt[:, o:o + 128], pt[:, o:o + 128], mk_sb[:]
                            )
                        ptmap[kt] = (pt, h)
                return ptmap

            prev = None
            for pair in range(PAIRS):
                b = pair // HL
                qcs = range(NQC)
                for qc in qcs:
                    q_sb = sbp.tile([128, QCH], f16, name="q", tag="q", bufs=4)
                    qdma = nc.scalar.dma_start if (pair == 0 and qc == 0) \
                        else nc.gpsimd.dma_start
                    qdma(out=q_sb[:], in_=qT[pair][:, qc * QCH:(qc + 1) * QCH])
                    if pair == 0:
                        emit_late_consts(qc)
                    pts = emit_qk_exp(b, q_sb, qc, 0)
                    if prev is not None:
                        ppair, pb, pqc, ppts = prev
                        ost = sbp.tile([128, QCH], f32, name="ost", tag="ost", bufs=3)
                        for qs in range(4):
                            pv_group(ppair, pb, pqc, ppts, ost, qs)
                    prev = (pair, b, qc, pts)
            ppair, pb, pqc, ppts = prev
            ost = sbp.tile([128, QCH], f32, name="ost", tag="ost", bufs=3)
            for qs in range(4):
                pv_group(ppair, pb, pqc, ppts, ost, qs)
    nc.compile()
    return nc


def _get_nc():
    if "nc" not in _CACHE:
        _CACHE["nc"] = _build()
    return _CACHE["nc"]


def _prep_inputs(q, k, v):
    """Build per-core input maps from full inputs."""
    q = np.asarray(q, dtype=np.float32)
    k = np.asarray(k, dtype=np.float32)
    v = np.asarray(v, dtype=np.float32)
    ones = np.ones((T, 1), dtype=np.float32)
    in_maps = []
    for c in range(NCORES):
        qTh = np.empty((PAIRS, 128, T), dtype=np.float32)
        for b in range(B):
            for hl in range(HL):
                qTh[b * HL + hl] = q[b, :, HL * c + hl, :].T
        kTh = np.empty((B, 128, T), dtype=np.float32)
        vph = np.empty((B, 128, KTS * 129), dtype=np.float16)
        for b in range(B):
            kTh[b] = k[b, :, c, :].T
            vcat = np.concatenate([v[b, :, c, :], ones], axis=1)  # [T, 129]
            vph[b] = (
                vcat.reshape(KTS, 128, 129).transpose(1, 0, 2).reshape(128, KTS * 129)
            ).astype(np.float16)
        mkm = (np.arange(128)[None, :] >= np.arange(128)[:, None]).astype(np.float16)
        in_maps.append({
            "qT": np.ascontiguousarray(qTh).astype(np.float16),
            "kT": np.ascontiguousarray(kTh).astype(np.float16),
            "vp": np.ascontiguousarray(vph),
            "mk": mkm,
        })
    return in_maps


def _assemble(results):
    full = np.empty((B, T, HQ * D), dtype=np.float32)
    for c in range(NCORES):
        res = results[c]["out"].reshape(PAIRS, 128, KTS, 128)
        for b in range(B):
            for hl in range(HL):
                h = HL * c + hl
                # [qrow, qt, d] -> [qt*128 + qrow, d]
                full[b, :, h * 128:(h + 1) * 128] = (
                    res[b * HL + hl].transpose(1, 0, 2).reshape(T, 128)
                )
    return full


def kernel(q, k, v):
    nc = _get_nc()
    in_maps = _prep_inputs(q, k, v)
    res = run_bass_kernel_spmd(nc, in_maps, core_ids=list(range(NCORES)))
    return _assemble(res.results)


# revision 41
# speedup vs baseline: 1.0105x; 1.0105x over previous
"""Causal GQA attention on 8 TRN2 NeuronCores (Bass/Tile).

Problem: B=2, Tq=Tk=2048, Hq=32, Hkv=8, D=128, fp32, causal softmax(QK^T/sqrt(D))V.

Sharding (tensor parallel over heads): core c handles q-heads [4c, 4c+4) and
kv-head c. Per core that is 8 independent (batch, head) single-head attentions
of Q[2048,128] x K[2048,128] -> V[2048,128]. No cross-core communication.

Per-core kernel layout (host pre-arranges everything into PE-friendly layouts):
  qT   [8, 128, 2048]  f32r  per (b, h_local): Q^T        (d on partitions)
  kT   [2, 128, 2048]  f32r  per batch:        K^T        (d on partitions)
  vp   [2, 128, 16*129] f16  per batch: V tiled [kt, 128, 129] partition-major,
                             col 128 of each tile = 1.0 (fused row-sum column)
  mk   [128, 128]      f16   upper-triangular (c >= p) 0/1 mask for diag tiles
  out  [8, 128, 16*128] f32  per pair: out[qrow, qt*128 + d]

Algorithm per (pair, q-chunk of 512) with k-tile pairs merged into [128,1024]
PSUM tiles (2 banks) so each ACT exp instruction covers 2 k-tiles:
  S^T[kt] = kT[kt-tile].T @ qT-chunk      (fp32r matmul, N=512, full PE rate)
  P^T[kt] = exp(SCALE * S^T[kt])          (ACT, fp16 out; causal: skip columns
                                           left of the diagonal, mask the
                                           diagonal 128x128 block on DVE)
  out[qs] = sum_kt P^T[kt][:, qs-block].T @ Vp[kt]   (fp16 matmul, N=129,
                                           PSUM-accumulated; col 128 = rowsum)
  out_norm = out[:, :128] * reciprocal(out[:, 128])  (DVE)

The Scalar engine (exp) is the bottleneck at ~83% of the kernel span, so the
whole schedule is built around keeping it saturated: PV matmul groups are
queued and drained between QK tile-pairs at a paced rate, so the PE always
alternates between feeding ACT new S^T tiles and chewing through PV backlog.
"""

from collections import deque

import numpy as np

import concourse.bass as bass
import concourse.tile as tile
from concourse import bacc, mybir
from concourse.bass_utils import run_bass_kernel_spmd

B, T, HQ, HKV, D = 2, 2048, 32, 8, 128
SCALE = 0.08838834764831845  # 1/sqrt(128)
NCORES = 8
HL = HQ // NCORES        # q heads per core
PAIRS = B * HL           # (batch, local head) pairs per core
QCH = 512                # q chunk = matmul moving free dim
NQC = T // QCH           # q chunks
KTS = T // 128           # k tiles
f32 = mybir.dt.float32
f32r = mybir.dt.float32r
f16 = mybir.dt.float16

_CACHE = {}


def _build():
    nc = bacc.Bacc("TRN2", target_bir_lowering=False, debug=False, num_devices=NCORES)
    qT = nc.dram_tensor("qT", [PAIRS, 128, T], f16, kind="ExternalInput").ap()
    kT = nc.dram_tensor("kT", [B, 128, T], f16, kind="ExternalInput").ap()
    vp = nc.dram_tensor("vp", [B, 128, KTS * 129], f16, kind="ExternalInput").ap()
    mk = nc.dram_tensor("mk", [128, 128], f16, kind="ExternalInput").ap()
    out = nc.dram_tensor("out", [PAIRS, 128, KTS * 128], f32, kind="ExternalOutput").ap()
    EXP = mybir.ActivationFunctionType.Exp

    # Total QK tile-pair slots and PV groups, for pacing the PV drain.
    slots_total = PAIRS * sum((4 * qc + 4) // 2 for qc in range(NQC))   # 160
    groups_total = PAIRS * NQC * 4                                       # 128

    with tile.TileContext(nc) as tc:
        with tc.tile_pool(name="const", bufs=1) as cst, \
             tc.tile_pool(name="sb", bufs=2) as sbp, \
             tc.tile_pool(name="pt", bufs=24) as ptp, \
             tc.tile_pool(name="st", bufs=3, space="PSUM") as stp, \
             tc.tile_pool(name="po", bufs=2, space="PSUM") as pop:
            # Latency-critical first loads ride the HWDGE sync queue;
            # bulk constants and output stores ride the gpsimd queue.
            kt0a = cst.tile([128, 256], f16, name="ktc0a", tag="ktc0a")
            nc.scalar.dma_start(out=kt0a[:], in_=kT[0][:, 0:256])
            mk_sb = cst.tile([128, 128], f16, name="mask", tag="mask")
            nc.sync.dma_start(out=mk_sb[:], in_=mk[:])
            # kT[0] is split across three tiles loaded in need-order, so the
            # first QK matmuls only wait on the small head pieces (tile-
            # granularity dependency tracking would otherwise stall them on
            # the full 1MB load).
            kt0b = cst.tile([128, 768], f16, name="ktc0b", tag="ktc0b")
            kt0c = cst.tile([128, 1024], f16, name="ktc0c", tag="ktc0c")
            vp0 = cst.tile([128, KTS * 129], f16, name="vpc0", tag="vpc0")
            kt1 = cst.tile([128, T], f16, name="ktc1", tag="ktc1")
            vp1 = cst.tile([128, KTS * 129], f16, name="vpc1", tag="vpc1")
            vps = [vp0, vp1]

            def ksrc(b, kt):
                if b == 1:
                    return kt1[:, kt * 128:(kt + 1) * 128]
                if kt < 2:
                    return kt0a[:, kt * 128:(kt + 1) * 128]
                if kt < 8:
                    return kt0b[:, (kt - 2) * 128:(kt - 1) * 128]
                return kt0c[:, (kt - 8) * 128:(kt - 7) * 128]

            def emit_late_consts(qc):
                # Bulk constant loads feed the gpsimd queue during pair 0,
                # in the order the pipeline needs them.
                if qc == 0:
                    nc.gpsimd.dma_start(out=kt0b[:], in_=kT[0][:, 256:1024])
                elif qc == 1:
                    nc.gpsimd.dma_start(out=kt0c[:], in_=kT[0][:, 1024:2048])
                    nc.gpsimd.dma_start(out=vp0[:], in_=vp[0])
                elif qc == 2:
                    nc.gpsimd.dma_start(out=kt1[:], in_=kT[1])
                else:
                    nc.gpsimd.dma_start(out=vp1[:], in_=vp[1])


            def pv_group(pair, b, qc, ptmap, ost, qs):
                gq = 4 * qc + qs
                ops = pop.tile([128, 129], f32, name="ops", tag="ops")
                for kt in range(gq + 1):
                    pt, h = ptmap[kt]
                    nc.tensor.matmul(
                        ops[:],
                        pt[:, h * 512 + qs * 128:h * 512 + (qs + 1) * 128],
                        vps[b][:, kt * 129:(kt + 1) * 129],
                        start=(kt == 0), stop=(kt == gq),
                    )
                rs = sbp.tile([128, 1], f32, name="rs", tag="rs", bufs=6)
                nc.vector.reciprocal(rs[:], ops[:, 128:129])
                nc.vector.tensor_scalar_mul(
                    ost[:, qs * 128:(qs + 1) * 128], ops[:, 0:128], rs[:]
                )
                if pair == PAIRS - 1 and qc == NQC - 1:
                    # final chunk: store each 128-col block as it normalizes
                    # so the stores overlap the remaining PV groups
                    nc.sync.dma_start(
                        out=out[pair][:, gq * 128:(gq + 1) * 128],
                        in_=ost[:, qs * 128:(qs + 1) * 128],
                    )
                elif qs == 3:
                    nc.sync.dma_start(
                        out=out[pair][:, qc * QCH:(qc + 1) * QCH], in_=ost[:]
                    )

            def emit_qk_exp(b, q_sb, qc, chunk_id):
                # k-tile pairs share one [128,1024] PSUM tile so one ACT exp
                # covers 2 k-tiles. The 4 partial (diagonal-crossing) k-tiles
                # are paired in reverse offset order -- (j3,j0) and (j2,j1),
                # low-offset tile in the second half -- so the single exp
                # range [c0:1024) wastes only 128 dead columns per chunk.
                nkt = 4 * qc + 4
                if qc == 0:
                    kt_pairs = [(3, 0), (2, 1)]
                    c0s = [384, 256]
                else:
                    kt_pairs = [(2 * a, 2 * a + 1) for a in range(2 * qc)] + \
                        [(4 * qc + 3, 4 * qc), (4 * qc + 2, 4 * qc + 1)]
                    c0s = [0] * (2 * qc) + [384, 256]
                ptmap = {}
                for a, (kta, ktb) in enumerate(kt_pairs):
                    st = stp.tile([128, 1024], f32, name="st", tag="st")
                    for h, kt in enumerate((kta, ktb)):
                        nc.tensor.matmul(
                            st[:, h * 512:(h + 1) * 512],
                            ksrc(b, kt),
                            q_sb[:],
                            start=True, stop=True,
                        )
                    pt = ptp.tile([128, 1024], f16, name="pt", tag="pt")
                    nc.scalar.activation(
                        pt[:, c0s[a]:1024], st[:, c0s[a]:1024], EXP, scale=SCALE
                    )
                    for h, kt in enumerate((kta, ktb)):
                        j = kt - 4 * qc
                        if j >= 0:   # diagonal 128x128 mask
                            o = h * 512 + 128 * j
                            nc.vector.tensor_mul(
                                pt[:, o:o + 128], pt[:, o:o + 128], mk_sb[:]
                            )
                        ptmap[kt] = (pt, h)
                return ptmap

            # Two-window PV split: a chunk's PV groups {0,1} run one chunk
            # later, {2,3} two chunks later, halving the PE's PV burst per
            # window so ACT's S^T backlog survives pair boundaries.
            hist = deque()   # (pair, b, qc, ptmap, ost)
            for pair in range(PAIRS):
                b = pair // HL
                for qc in range(NQC):
                    q_sb = sbp.tile([128, QCH], f16, name="q", tag="q", bufs=4)
                    qdma = nc.scalar.dma_start if (pair == 0 and qc == 0) \
                        else nc.gpsimd.dma_start
                    qdma(out=q_sb[:], in_=qT[pair][:, qc * QCH:(qc + 1) * QCH])
                    if pair == 0:
                        emit_late_consts(qc)
                    pts = emit_qk_exp(b, q_sb, qc, 0)
                    if len(hist) == 2:
                        g = hist.popleft()
                        pv_group(g[0], g[1], g[2], g[3], g[4], 2)
                        pv_group(g[0], g[1], g[2], g[3], g[4], 3)
                    if hist:
                        g = hist[-1]
                        pv_group(g[0], g[1], g[2], g[3], g[4], 0)
                        pv_group(g[0], g[1], g[2], g[3], g[4], 1)
                    ost = sbp.tile([128, QCH], f32, name="ost", tag="ost", bufs=4)
                    hist.append((pair, b, qc, pts, ost))
            g = hist.popleft()
            pv_group(g[0], g[1], g[2], g[3], g[4], 2)
            pv_group(g[0], g[1], g[2], g[3], g[4], 3)
            g = hist.popleft()
            for qs in range(4):
                pv_group(g[0], g[1], g[2], g[3], g[4], qs)
    nc.compile()
    return nc


def _get_nc():
    if "nc" not in _CACHE:
        _CACHE["nc"] = _build()
    return _CACHE["nc"]


def _prep_inputs(q, k, v):
    """Build per-core input maps from full inputs."""
    q = np.asarray(q, dtype=np.float32)
    k = np.asarray(k, dtype=np.float32)
    v = np.asarray(v, dtype=np.float32)
    ones = np.ones((T, 1), dtype=np.float32)
    in_maps = []
    for c in range(NCORES):
        qTh = np.empty((PAIRS, 128, T), dtype=np.float32)
        for b in range(B):
            for hl in range(HL):
                qTh[b * HL + hl] = q[b, :, HL * c + hl, :].T
        kTh = np.empty((B, 128, T), dtype=np.float32)
        vph = np.empty((B, 128, KTS * 129), dtype=np.float16)
        for b in range(B):
            kTh[b] = k[b, :, c, :].T
            vcat = np.concatenate([v[b, :, c, :], ones], axis=1)  # [T, 129]
            vph[b] = (
                vcat.reshape(KTS, 128, 129).transpose(1, 0, 2).reshape(128, KTS * 129)
            ).astype(np.float16)
        mkm = (np.arange(128)[None, :] >= np.arange(128)[:, None]).astype(np.float16)
        in_maps.append({
            "qT": np.ascontiguousarray(qTh).astype(np.float16),
            "kT": np.ascontiguousarray(kTh).astype(np.float16),
            "vp": np.ascontiguousarray(vph),
            "mk": mkm,
        })
    return in_maps


def _assemble(results):
    full = np.empty((B, T, HQ * D), dtype=np.float32)
    for c in range(NCORES):
        res = results[c]["out"].reshape(PAIRS, 128, KTS, 128)
        for b in range(B):
            for hl in range(HL):
                h = HL * c + hl
                # [qrow, qt, d] -> [qt*128 + qrow, d]
                full[b, :, h * 128:(h + 1) * 128] = (
                    res[b * HL + hl].transpose(1, 0, 2).reshape(T, 128)
                )
    return full


def kernel(q, k, v):
    nc = _get_nc()
    in_maps = _prep_inputs(q, k, v)
    res = run_bass_kernel_spmd(nc, in_maps, core_ids=list(range(NCORES)))
    return _assemble(res.results)
